# revision 23
# baseline (speedup 1.0000x reference)
"""Trainium2 Bass kernel for nn_BaselineGPT (sliding-window GQA attention block).

Sharding: 8 cores = 2 batches x 4 sequence chunks of 512 queries.
Each core computes its 512 output rows end-to-end (QKV proj, RMS norm, RoPE,
windowed GQA attention, output proj).  KV halo of 256 rows comes with the
chunk; chunk 0's missing halo is masked via a -30000 exp-bias.  Pair-head
mixing is folded into Wo on the host.

v2 layout: Q^T and K^T are produced directly by weight-stationary matmuls
(lhsT = W, rhs = x^T), so no PE transposes are needed.  RMS norm runs in the
[d, seq] layout: sum-of-squares via a masked-ones matmul (partition reduce),
broadcast back across partitions with gpsimd partition_broadcast.  RoPE uses
sign-folded cos/sin tables in bf16.  Scores for the two KV groups of a pair
run concurrently in the PE array via row tiling (base partitions 0 / 64).
"""

import math
from contextlib import ExitStack

import numpy as np

import concourse.bass as bass
from concourse import bacc
import concourse.mybir as mybir
import concourse.tile as tile

B, S, DIM = 2, 2048, 1024
H, KVH, HD = 16, 4, 64
WINDOW = 256
ROPE_BASE = 10000.0
EPS = 1e-6

NQ = 512          # queries per core
NK = 768          # kv rows per core (incl 256 halo)
NCORES = 8
F32 = mybir.dt.float32
BF16 = mybir.dt.bfloat16

_BUILT = None


def _build(debug=False):
    nc = bacc.Bacc(None)

    xt = nc.declare_dram_parameter("xt", [DIM, NK], BF16, isOutput=False)
    wq = nc.declare_dram_parameter("wq", [DIM, DIM], BF16, isOutput=False)
    wk = nc.declare_dram_parameter("wk", [DIM, KVH * HD], BF16, isOutput=False)
    wv = nc.declare_dram_parameter("wv", [DIM, KVH * HD], BF16, isOutput=False)
    wo = nc.declare_dram_parameter("wo", [DIM, DIM], BF16, isOutput=False)
    ropec = nc.declare_dram_parameter("ropec", [128, NK], BF16, isOutput=False)
    ropes = nc.declare_dram_parameter("ropes", [128, NK], BF16, isOutput=False)
    kbt = nc.declare_dram_parameter("kbt", [128, 6], F32, isOutput=False)
    qg = nc.declare_dram_parameter("qg", [1, 2 * H], F32, isOutput=False)
    m0 = nc.declare_dram_parameter("m0", [128, 512], BF16, isOutput=False)
    m2 = nc.declare_dram_parameter("m2", [128, 512], BF16, isOutput=False)
    out = nc.declare_dram_parameter("out", [NQ, DIM], F32, isOutput=True)
    if debug:
        d_ktp = nc.declare_dram_parameter("d_ktp", [256, NK], BF16, isOutput=True)
        d_qtp = nc.declare_dram_parameter("d_qtp", [256, 4 * NQ], BF16, isOutput=True)
        d_v = nc.declare_dram_parameter("d_v", [128, 6 * KVH * (HD + 1)], BF16, isOutput=True)
        d_yt = nc.declare_dram_parameter("d_yt", [128, 8 * NQ], BF16, isOutput=True)
        d_att = nc.declare_dram_parameter("d_att", [128, 3 * 1024], BF16, isOutput=True)
        d_sq = nc.declare_dram_parameter("d_sq", [128, NK], BF16, isOutput=True)
        d_invb = nc.declare_dram_parameter("d_invb", [128, NK], F32, isOutput=True)
        d_kn = nc.declare_dram_parameter("d_kn", [128, NK], BF16, isOutput=True)
        d_t1 = nc.declare_dram_parameter("d_t1", [128, NK], BF16, isOutput=True)
        d_u = nc.declare_dram_parameter("d_u", [128, NK], BF16, isOutput=True)
        d_t2 = nc.declare_dram_parameter("d_t2", [128, NK], BF16, isOutput=True)
        d_ssq = nc.declare_dram_parameter("d_ssq", [128, NK], F32, isOutput=True)
        d_inv = nc.declare_dram_parameter("d_inv", [128, NK], F32, isOutput=True)
        d_invr = nc.declare_dram_parameter("d_invr", [128, NK], F32, isOutput=True)
        d_hm = nc.declare_dram_parameter("d_hm", [128, 65], BF16, isOutput=True)

    with tile.TileContext(nc) as tc, ExitStack() as ctx:
        const = ctx.enter_context(tc.tile_pool(name="const", bufs=1))
        big = ctx.enter_context(tc.tile_pool(name="big", bufs=1))
        sqp = ctx.enter_context(tc.tile_pool(name="sqp", bufs=2))
        tp = ctx.enter_context(tc.tile_pool(name="tp", bufs=2))
        invp = ctx.enter_context(tc.tile_pool(name="invp", bufs=2))
        attp = ctx.enter_context(tc.tile_pool(name="attp", bufs=4))
        rp = ctx.enter_context(tc.tile_pool(name="rp", bufs=3))
        obp = ctx.enter_context(tc.tile_pool(name="obp", bufs=2))
        psb = ctx.enter_context(tc.tile_pool(name="psb", bufs=4, space="PSUM"))
        psa = ctx.enter_context(tc.tile_pool(name="psa", bufs=2, space="PSUM"))
        pss = ctx.enter_context(tc.tile_pool(name="pss", bufs=2, space="PSUM"))

        # ---- small constants (scalar HWDGE queue, first) ----
        ropec_sb = const.tile([128, NK], BF16, tag="ropec")
        nc.scalar.dma_start(out=ropec_sb, in_=ropec[:, :])
        ropes_sb = const.tile([128, NK], BF16, tag="ropes")
        nc.scalar.dma_start(out=ropes_sb, in_=ropes[:, :])
        kbt_sb = const.tile([128, 6], F32, tag="kbt")
        nc.scalar.dma_start(out=kbt_sb, in_=kbt[:, :])
        qg_sb = const.tile([128, 2 * H], F32, tag="qg")
        nc.scalar.dma_start(out=qg_sb, in_=qg[0:1, :].to_broadcast((128, 2 * H)))
        m0_sb = const.tile([128, 512], BF16, tag="m0")
        nc.scalar.dma_start(out=m0_sb, in_=m0[:, :])
        m2_sb = const.tile([128, 512], BF16, tag="m2")
        nc.scalar.dma_start(out=m2_sb, in_=m2[:, :])

        eps_t = const.tile([128, 1], F32, tag="eps")
        nc.vector.memset(eps_t, EPS)
        ones_f = const.tile([128, 128], F32, tag="onesf")
        nc.vector.memset(ones_f, 1.0)
        hmask = const.tile([128, 65], BF16, tag="hmask")
        nc.vector.memset(hmask, 0.0)
        nc.vector.memset(hmask[0:64, 0:1], 1.0)
        nc.vector.memset(hmask[64:128, 64:65], 1.0)

        # ---- weights / activations (sync queue: wk, wv, xt; scalar: wq; gpsimd: wo)
        xt_sb, wq_sb, wk_sb, wv_sb, wo_sb = [], [], [], [], []
        for kt in range(8):
            sl = slice(kt * 128, kt * 128 + 128)
            t = big.tile([128, KVH * HD], BF16, tag=f"wk{kt}", name=f"wk{kt}")
            nc.sync.dma_start(out=t, in_=wk[sl, :])
            wk_sb.append(t)
            t = big.tile([128, KVH * HD], BF16, tag=f"wv{kt}", name=f"wv{kt}")
            nc.sync.dma_start(out=t, in_=wv[sl, :])
            wv_sb.append(t)
            t = big.tile([128, NK], BF16, tag=f"xt{kt}", name=f"xt{kt}")
            nc.sync.dma_start(out=t, in_=xt[sl, :])
            xt_sb.append(t)
        for kt in range(8):
            sl = slice(kt * 128, kt * 128 + 128)
            t = big.tile([128, DIM], BF16, tag=f"wq{kt}", name=f"wq{kt}")
            nc.scalar.dma_start(out=t, in_=wq[sl, :])
            wq_sb.append(t)
            t = big.tile([128, DIM], BF16, tag=f"wo{kt}", name=f"wo{kt}")
            nc.gpsimd.dma_start(out=t, in_=wo[sl, :])
            wo_sb.append(t)

        # ---- persistent SBUF ----
        ktp = [
            big.tile([128, NK], BF16, tag=f"ktp{j}", name=f"ktp{j}") for j in range(2)
        ]
        qtp = [
            big.tile([128, 4, NQ], BF16, tag=f"qtp{j}", name=f"qtp{j}")
            for j in range(2)
        ]
        v_sb = big.tile([128, 6, KVH, HD + 1], BF16, tag="v")
        nc.vector.memset(v_sb[:, :, :, HD : HD + 1], 1.0)
        yt_sb = big.tile([128, 8, NQ], BF16, tag="yt")

        def rope(v, dst, w, off=0, dbg=False):
            """v: [128, w] bf16 normalized (d-partition layout), covering kv
            positions [off, off+w).  Writes the rotated result to dst
            ([128, w], same-start)."""
            t1 = tp.tile([128, NK], BF16, tag="t1")
            nc.vector.tensor_mul(out=t1[:, 0:w], in0=v, in1=ropec_sb[:, off : off + w])
            u = tp.tile([128, NK], BF16, tag="u")
            nc.vector.tensor_mul(out=u[:, 0:w], in0=v, in1=ropes_sb[:, off : off + w])
            t2 = tp.tile([128, NK], BF16, tag="t2")
            for qtr in range(4):
                sp = (qtr ^ 1) * 32
                nc.vector.tensor_copy(
                    out=t2[qtr * 32 : qtr * 32 + 32, 0:w],
                    in_=u[sp : sp + 32, 0:w],
                )
            if dbg:
                nc.sync.dma_start(out=d_t1[:, 0:w], in_=t1[:, 0:w])
                nc.sync.dma_start(out=d_u[:, 0:w], in_=u[:, 0:w])
                nc.sync.dma_start(out=d_t2[:, 0:w], in_=t2[:, 0:w])
            nc.vector.tensor_add(out=dst, in0=t1[:, 0:w], in1=t2[:, 0:w])

        # ---------------- K projection: kt-outer over 4 psum banks ----------
        psK = [psb.tile([128, 512], F32, tag="p512", name=f"psK{i}") for i in range(4)]
        for kt in range(8):
            for jp in range(2):
                for nh in range(2):
                    nc.tensor.matmul(
                        out=psK[jp * 2 + nh][:, 0:384],
                        lhsT=wk_sb[kt][:, jp * 128 : jp * 128 + 128],
                        rhs=xt_sb[kt][:, nh * 384 : nh * 384 + 384],
                        start=(kt == 0),
                        stop=(kt == 7),
                        skip_group_check=True,
                    )

        # ---- K norm + rope -> ktp[jp] ----
        for jp in range(2):
            sqK = sqp.tile([128, NK], BF16, tag="sq")
            for nh in range(2):
                nc.scalar.activation(
                    out=sqK[:, nh * 384 : nh * 384 + 384],
                    in_=psK[jp * 2 + nh][:, 0:384],
                    func=mybir.ActivationFunctionType.Square,
                )
            ssq = [pss.tile([65, 512], F32, tag="ps2", name=f"ssqK{i}") for i in range(2)]
            for nh in range(2):
                nc.tensor.matmul(
                    out=ssq[nh][0:65, 0:384],
                    lhsT=hmask,
                    rhs=sqK[:, nh * 384 : nh * 384 + 384],
                    start=True,
                    stop=True,
                )
            inv = invp.tile([128, NK], F32, tag="inv")
            for nh in range(2):
                for row in range(2):
                    nc.scalar.activation(
                        out=inv[row * 64 : row * 64 + 1, nh * 384 : nh * 384 + 384],
                        in_=ssq[nh][row * 64 : row * 64 + 1, 0:384],
                        func=mybir.ActivationFunctionType.Sqrt,
                        bias=eps_t[row * 64 : row * 64 + 1, :],
                        scale=1.0 / HD,
                    )
            invr = invp.tile([128, NK], F32, tag="invr")
            nc.vector.reciprocal(out=invr[0:1, :], in_=inv[0:1, :])
            nc.vector.reciprocal(out=invr[64:65, :], in_=inv[64:65, :])
            invb = invp.tile([128, NK], F32, tag="invb")
            for nh in range(2):
                ibp = pss.tile([128, 512], F32, tag="ps2", name="ibpK")
                nc.tensor.matmul(
                    out=ibp[0:64, 0:384],
                    lhsT=ones_f[0:1, 0:64],
                    rhs=invr[0:1, nh * 384 : nh * 384 + 384],
                    start=True,
                    stop=True,
                )
                nc.tensor.matmul(
                    out=ibp[64:128, 0:384],
                    lhsT=ones_f[64:65, 0:64],
                    rhs=invr[64:65, nh * 384 : nh * 384 + 384],
                    start=True,
                    stop=True,
                )
                nc.scalar.copy(
                    out=invb[:, nh * 384 : nh * 384 + 384], in_=ibp[:, 0:384]
                )
            kn = tp.tile([128, NK], BF16, tag="kn")
            for nh in range(2):
                nc.vector.tensor_mul(
                    out=kn[:, nh * 384 : nh * 384 + 384],
                    in0=psK[jp * 2 + nh][:, 0:384],
                    in1=invb[:, nh * 384 : nh * 384 + 384],
                )
            if debug and jp == 0:
                ssq_cp = sqp.tile([128, NK], F32, tag="dbgssq", name="ssq_cp")
                for nh in range(2):
                    nc.scalar.copy(
                        out=ssq_cp[0:65, nh * 384 : nh * 384 + 384],
                        in_=ssq[nh][0:65, 0:384],
                    )
                nc.sync.dma_start(out=d_ssq[0:65, :], in_=ssq_cp[0:65, :])
                nc.sync.dma_start(out=d_inv[0:1, :], in_=inv[0:1, :])
                nc.sync.dma_start(out=d_inv[64:65, :], in_=inv[64:65, :])
                nc.sync.dma_start(out=d_invr[0:1, :], in_=invr[0:1, :])
                nc.sync.dma_start(out=d_invr[64:65, :], in_=invr[64:65, :])
                nc.sync.dma_start(out=d_hm[:, :], in_=hmask)
                nc.sync.dma_start(out=d_sq[:, :], in_=sqK)
                nc.sync.dma_start(out=d_invb[:, :], in_=invb)
                nc.sync.dma_start(out=d_kn[:, :], in_=kn)
            rope(kn, ktp[jp], NK, dbg=(debug and jp == 0))

        # ---------------- V projection: st-inner, double-buffered ----------
        for st in range(6):
            pv = psa.tile([128, 512], F32, tag="paux")
            for kt in range(8):
                nc.tensor.matmul(
                    out=pv[:, 0 : KVH * HD],
                    lhsT=xt_sb[kt][:, st * 128 : st * 128 + 128],
                    rhs=wv_sb[kt],
                    start=(kt == 0),
                    stop=(kt == 7),
                )
            nc.vector.tensor_copy(
                out=v_sb[:, st, :, 0:HD],
                in_=pv[:, 0 : KVH * HD].rearrange("p (g d) -> p g d", d=HD),
            )

        # ---------------- Q projection: e-tile inner ----------
        for t in range(8):
            pq = psb.tile([128, 512], F32, tag="p512")
            for kt in range(8):
                nc.tensor.matmul(
                    out=pq,
                    lhsT=wq_sb[kt][:, t * 128 : t * 128 + 128],
                    rhs=xt_sb[kt][:, 256:768],
                    start=(kt == 0),
                    stop=(kt == 7),
                )
            sqQ = sqp.tile([128, NK], BF16, tag="sq")
            nc.scalar.activation(
                out=sqQ[:, 0:512],
                in_=pq,
                func=mybir.ActivationFunctionType.Square,
            )
            ssq = pss.tile([65, 512], F32, tag="ps2")
            nc.tensor.matmul(
                out=ssq[0:65, :], lhsT=hmask, rhs=sqQ[:, 0:512], start=True, stop=True
            )
            inv = invp.tile([128, NK], F32, tag="inv")
            for row in range(2):
                # 1/(qg*sqrt(m+eps)) = 1/sqrt(qg^2*m + qg^2*eps); qg_sb holds
                # qg^2/HD and qg^2*eps per head (broadcast to all partitions)
                p0 = row * 64
                h2 = 2 * (2 * t + row)
                nc.scalar.activation(
                    out=inv[p0 : p0 + 1, 0:512],
                    in_=ssq[p0 : p0 + 1, :],
                    func=mybir.ActivationFunctionType.Sqrt,
                    bias=qg_sb[p0 : p0 + 1, h2 + 1 : h2 + 2],
                    scale=qg_sb[p0 : p0 + 1, h2 : h2 + 1],
                )
            invr = invp.tile([128, NK], F32, tag="invr")
            nc.vector.reciprocal(out=invr[0:1, 0:512], in_=inv[0:1, 0:512])
            nc.vector.reciprocal(out=invr[64:65, 0:512], in_=inv[64:65, 0:512])
            invb = invp.tile([128, NK], F32, tag="invb")
            ibp = pss.tile([128, 512], F32, tag="ps2", name="ibpQ")
            nc.tensor.matmul(
                out=ibp[0:64, :],
                lhsT=ones_f[0:1, 0:64],
                rhs=invr[0:1, 0:512],
                start=True,
                stop=True,
            )
            nc.tensor.matmul(
                out=ibp[64:128, :],
                lhsT=ones_f[64:65, 0:64],
                rhs=invr[64:65, 0:512],
                start=True,
                stop=True,
            )
            nc.scalar.copy(out=invb[:, 0:512], in_=ibp)
            qn = tp.tile([128, NK], BF16, tag="kn")
            nc.vector.tensor_mul(out=qn[:, 0:512], in0=pq, in1=invb[:, 0:512])
            # rotate then scatter the two heads into the qtp pair layout
            g = t // 2
            jp, pb = g // 2, (g % 2) * 64
            hh0 = 2 * (t % 2)
            qrot = tp.tile([128, NK], BF16, tag="qrot")
            rope(qn[:, 0:512], qrot[:, 0:512], 512, off=256)
            nc.vector.tensor_copy(
                out=qtp[jp][pb : pb + 64, hh0, :], in_=qrot[0:64, 0:512]
            )
            nc.vector.tensor_copy(
                out=qtp[jp][pb : pb + 64, hh0 + 1, :], in_=qrot[64:128, 0:512]
            )

        # ---------------- attention ----------
        def attention(jp, qb):
            atts = []
            for t in range(3):
                kw = slice(qb * 128 + t * 128, qb * 128 + t * 128 + 128)
                att = attp.tile([128, 1024], BF16, tag="att")
                kc = qb + t
                for half in range(2):
                    ps_ = psb.tile([128, 512], F32, tag="p512")
                    nc.tensor.matmul(
                        out=ps_,
                        lhsT=ktp[jp][half * 64 : half * 64 + 64, kw],
                        rhs=qtp[jp][
                            half * 64 : half * 64 + 64, :, qb * 128 : qb * 128 + 128
                        ],
                        start=True,
                        stop=True,
                    )
                    nc.scalar.activation(
                        out=att[:, half * 512 : half * 512 + 512],
                        in_=ps_,
                        func=mybir.ActivationFunctionType.Exp,
                        bias=kbt_sb[:, kc : kc + 1],
                    )
                if t == 0:
                    for half in range(2):
                        nc.vector.tensor_mul(
                            out=att[:, half * 512 : half * 512 + 512],
                            in0=att[:, half * 512 : half * 512 + 512],
                            in1=m0_sb,
                        )
                if t == 2:
                    for half in range(2):
                        nc.vector.tensor_mul(
                            out=att[:, half * 512 : half * 512 + 512],
                            in0=att[:, half * 512 : half * 512 + 512],
                            in1=m2_sb,
                        )
                if debug and jp == 0 and qb == 1:
                    nc.sync.dma_start(
                        out=d_att[:, t * 1024 : t * 1024 + 1024], in_=att
                    )
                atts.append(att)
            for half in range(2):
                g = 2 * jp + half
                psy = psa.tile([128, 512], F32, tag="paux")
                for t in range(3):
                    nc.tensor.matmul(
                        out=psy[0:65, :],
                        lhsT=v_sb[:, qb + t, g, :],
                        rhs=atts[t][:, half * 512 : half * 512 + 512],
                        start=(t == 0),
                        stop=(t == 2),
                    )
                dn = rp.tile([128, 512], F32, tag="dn")
                nc.vector.tensor_copy(out=dn[64:65, :], in_=psy[64:65, :])
                rbp = psb.tile([128, 512], F32, tag="p512", name="rbp")
                nc.tensor.matmul(
                    out=rbp,
                    lhsT=ones_f[64:65, 0:128],
                    rhs=dn[64:65, :],
                    start=True,
                    stop=True,
                )
                rb = rp.tile([128, 512], F32, tag="rb")
                nc.vector.reciprocal_approx_fast(out=rb, in_=rbp)
                for hh in range(4):
                    h = g * 4 + hh
                    pair, lo = h // 2, (h % 2) * 64
                    nc.vector.tensor_mul(
                        out=yt_sb[lo : lo + 64, pair, qb * 128 : qb * 128 + 128],
                        in0=psy[0:64, hh * 128 : hh * 128 + 128],
                        in1=rb[lo : lo + 64, hh * 128 : hh * 128 + 128],
                    )

        if debug:
            for j in range(2):
                nc.sync.dma_start(out=d_ktp[j * 128 : j * 128 + 128, :], in_=ktp[j])
                nc.sync.dma_start(
                    out=d_qtp[j * 128 : j * 128 + 128, :],
                    in_=qtp[j].rearrange("p h q -> p (h q)"),
                )
            nc.sync.dma_start(out=d_v[:, :], in_=v_sb.rearrange("p a b c -> p (a b c)"))

        for qb in range(4):
            attention(0, qb)
        for qb in range(4):
            attention(1, qb)
            # ---- output projection for this query block ----
            ob = obp.tile([128, DIM], F32, tag="ob")
            for half in range(2):
                po = psb.tile([128, 512], F32, tag="p512")
                for p in range(8):
                    nc.tensor.matmul(
                        out=po,
                        lhsT=yt_sb[:, p, qb * 128 : qb * 128 + 128],
                        rhs=wo_sb[p][:, half * 512 : half * 512 + 512],
                        start=(p == 0),
                        stop=(p == 7),
                    )
                nc.scalar.copy(out=ob[:, half * 512 : half * 512 + 512], in_=po)
            nc.sync.dma_start(out=out[qb * 128 : qb * 128 + 128, :], in_=ob)
        if debug:
            nc.sync.dma_start(out=d_yt[:, :], in_=yt_sb.rearrange("p h q -> p (h q)"))

    nc.finalize()
    return nc


def _host_inputs(x, Wq, Wk, Wv, Wo, q_gain, pair_mix):
    """Build the 8 per-core input maps."""
    x = np.asarray(x, np.float32)
    Wq = np.asarray(Wq, np.float32)
    Wk = np.asarray(Wk, np.float32)
    Wv = np.asarray(Wv, np.float32)
    Wo = np.asarray(Wo, np.float32)
    q_gain = np.asarray(q_gain, np.float32)
    pair_mix = np.asarray(pair_mix, np.float32)

    # fold pair mixing into Wo:  out = y_mix @ Wo.T,  y_mix = y @ M.T  =>  Wo' = Wo @ M
    M = np.zeros((DIM, DIM), np.float32)
    eye = np.eye(HD, dtype=np.float32)
    for p in range(H // 2):
        for o in range(2):
            for i in range(2):
                ho, hi = 2 * p + o, 2 * p + i
                M[ho * HD : ho * HD + HD, hi * HD : hi * HD + HD] = (
                    pair_mix[p, o, i] * eye
                )
    woT = np.ascontiguousarray((Wo @ M).T)

    wqT = np.ascontiguousarray(Wq.T)
    wkT = np.ascontiguousarray(Wk.T)
    wvT = np.ascontiguousarray(Wv.T)
    qg2 = (q_gain / math.sqrt(HD)).astype(np.float32) ** 2
    qgt = np.zeros((1, 2 * H), np.float32)
    qgt[0, 0::2] = 1.0 / (HD * qg2)
    qgt[0, 1::2] = EPS / qg2

    inv_freq = 1.0 / (ROPE_BASE ** (np.arange(0, HD, 2, dtype=np.float32) / HD))

    ql = np.arange(128)
    m0_ = (ql[:, None] >= ql[None, :] + 1).astype(np.float32)  # kl >= ql+1
    m2_ = (ql[:, None] <= ql[None, :]).astype(np.float32)      # kl <= ql
    m0t = np.ascontiguousarray(np.tile(m0_, (1, 4)))
    m2t = np.ascontiguousarray(np.tile(m2_, (1, 4)))

    import ml_dtypes

    bf = ml_dtypes.bfloat16
    wqT, wkT, wvT, woT = (a.astype(bf) for a in (wqT, wkT, wvT, woT))
    m0t, m2t = m0t.astype(bf), m2t.astype(bf)

    # rope tables in [d-partition, pos] layout, head block repeated twice;
    # rows 0-31: +sin, rows 32-63: -sin (sign folded)
    r = np.arange(128)
    freqs_row = inv_freq[r % 32]                       # [128]
    sign = np.where((r % 64) < 32, -1.0, 1.0).astype(np.float32)

    in_maps = []
    for core in range(NCORES):
        b, c = core // 4, core % 4
        ks = 512 * c - 256
        xc = np.zeros((NK, DIM), np.float32)
        lo = max(0, ks)
        xc[lo - ks :] = x[b, lo : ks + NK]
        pos = (ks + np.arange(NK, dtype=np.float32))[None, :]    # [1, NK]
        ang = freqs_row[:, None] * pos                           # [128, NK]
        ropec_ = np.cos(ang).astype(bf)
        ropes_ = (np.sin(ang) * sign[:, None]).astype(bf)
        kbt_ = np.zeros((128, 6), np.float32)
        if c == 0:
            kbt_[:, 0:2] = -30000.0
        in_maps.append(
            {
                "xt": np.ascontiguousarray(xc.T).astype(bf),
                "wq": wqT,
                "wk": wkT,
                "wv": wvT,
                "wo": woT,
                "ropec": np.ascontiguousarray(ropec_),
                "ropes": np.ascontiguousarray(ropes_),
                "kbt": kbt_,
                "qg": qgt,
                "m0": m0t,
                "m2": m2t,
            }
        )
    return in_maps


def kernel(x, Wq, Wk, Wv, Wo, q_gain, pair_mix):
    global _BUILT
    from concourse.bass_utils import run_bass_kernel_spmd

    if _BUILT is None:
        _BUILT = _build()
    in_maps = _host_inputs(x, Wq, Wk, Wv, Wo, q_gain, pair_mix)
    res = run_bass_kernel_spmd(_BUILT, in_maps, list(range(NCORES)))
    out = np.empty((B, S, DIM), np.float32)
    for core in range(NCORES):
        b, c = core // 4, core % 4
        out[b, 512 * c : 512 * c + 512, :] = res.results[core]["out"]
    return out


# revision 30
# speedup vs baseline: 1.7695x; 1.7695x over previous
"""Trainium2 Bass kernel for nn_BaselineGPT (sliding-window GQA attention block).

Sharding: 8 cores = 2 batches x 4 sequence chunks of 512 queries.
Each core computes its 512 output rows end-to-end (QKV proj, RMS norm, RoPE,
windowed GQA attention, output proj).  KV halo of 256 rows comes with the
chunk; chunk 0's missing halo is masked via a -30000 exp-bias.  Pair-head
mixing is folded into Wo on the host.

v2 layout: Q^T and K^T are produced directly by weight-stationary matmuls
(lhsT = W, rhs = x^T), so no PE transposes are needed.  RMS norm runs in the
[d, seq] layout: sum-of-squares via a masked-ones matmul (partition reduce),
broadcast back across partitions with gpsimd partition_broadcast.  RoPE uses
sign-folded cos/sin tables in bf16.  Scores for the two KV groups of a pair
run concurrently in the PE array via row tiling (base partitions 0 / 64).
"""

import math
from contextlib import ExitStack

import numpy as np

import concourse.bass as bass
from concourse import bacc
import concourse.mybir as mybir
import concourse.tile as tile

B, S, DIM = 2, 2048, 1024
H, KVH, HD = 16, 4, 64
WINDOW = 256
ROPE_BASE = 10000.0
EPS = 1e-6

NQ = 512          # queries per core
NK = 768          # kv rows per core (incl 256 halo)
NCORES = 8
F32 = mybir.dt.float32
BF16 = mybir.dt.bfloat16

_BUILT = None


def _build(debug=False):
    nc = bacc.Bacc(None)

    xt = nc.declare_dram_parameter("xt", [DIM, NK], BF16, isOutput=False)
    wq = nc.declare_dram_parameter("wq", [DIM, DIM], BF16, isOutput=False)
    wk = nc.declare_dram_parameter("wk", [DIM, KVH * HD], BF16, isOutput=False)
    wv = nc.declare_dram_parameter("wv", [DIM, KVH * HD], BF16, isOutput=False)
    wo = nc.declare_dram_parameter("wo", [DIM, DIM], BF16, isOutput=False)
    ropec = nc.declare_dram_parameter("ropec", [128, NK], BF16, isOutput=False)
    ropes = nc.declare_dram_parameter("ropes", [128, NK], BF16, isOutput=False)
    kbt = nc.declare_dram_parameter("kbt", [128, 6], F32, isOutput=False)
    qg = nc.declare_dram_parameter("qg", [1, 2 * H], F32, isOutput=False)
    m0 = nc.declare_dram_parameter("m0", [128, 512], BF16, isOutput=False)
    m2 = nc.declare_dram_parameter("m2", [128, 512], BF16, isOutput=False)
    out = nc.declare_dram_parameter("out", [NQ, DIM], F32, isOutput=True)
    if debug:
        d_ktp = nc.declare_dram_parameter("d_ktp", [256, NK], BF16, isOutput=True)
        d_qtp = nc.declare_dram_parameter("d_qtp", [256, 4 * NQ], BF16, isOutput=True)
        d_v = nc.declare_dram_parameter("d_v", [128, 6 * KVH * 2 * HD], BF16, isOutput=True)
        d_yt = nc.declare_dram_parameter("d_yt", [128, 8 * NQ], BF16, isOutput=True)
        d_att = nc.declare_dram_parameter("d_att", [128, 3 * 1024], BF16, isOutput=True)
        d_sq = nc.declare_dram_parameter("d_sq", [128, NK], BF16, isOutput=True)
        d_invb = nc.declare_dram_parameter("d_invb", [128, NK], F32, isOutput=True)
        d_kn = nc.declare_dram_parameter("d_kn", [128, NK], BF16, isOutput=True)
        d_t1 = nc.declare_dram_parameter("d_t1", [128, NK], BF16, isOutput=True)
        d_u = nc.declare_dram_parameter("d_u", [128, NK], BF16, isOutput=True)
        d_t2 = nc.declare_dram_parameter("d_t2", [128, NK], BF16, isOutput=True)
        d_ssq = nc.declare_dram_parameter("d_ssq", [128, NK], F32, isOutput=True)
        d_inv = nc.declare_dram_parameter("d_inv", [128, NK], F32, isOutput=True)
        d_invr = nc.declare_dram_parameter("d_invr", [128, NK], F32, isOutput=True)
        d_hm = nc.declare_dram_parameter("d_hm", [128, 65], BF16, isOutput=True)

    with tile.TileContext(nc) as tc, ExitStack() as ctx:
        const = ctx.enter_context(tc.tile_pool(name="const", bufs=1))
        big = ctx.enter_context(tc.tile_pool(name="big", bufs=1))
        sqp = ctx.enter_context(tc.tile_pool(name="sqp", bufs=2))
        tp = ctx.enter_context(tc.tile_pool(name="tp", bufs=2))
        invp = ctx.enter_context(tc.tile_pool(name="invp", bufs=2))
        attp = ctx.enter_context(tc.tile_pool(name="attp", bufs=6))
        rp = ctx.enter_context(tc.tile_pool(name="rp", bufs=3))
        obp = ctx.enter_context(tc.tile_pool(name="obp", bufs=2))
        psb = ctx.enter_context(tc.tile_pool(name="psb", bufs=4, space="PSUM"))
        psa = ctx.enter_context(tc.tile_pool(name="psa", bufs=2, space="PSUM"))
        pss = ctx.enter_context(tc.tile_pool(name="pss", bufs=2, space="PSUM"))

        # ---- small constants (scalar HWDGE queue, first) ----
        ropec_sb = const.tile([128, NK], BF16, tag="ropec")
        nc.scalar.dma_start(out=ropec_sb, in_=ropec[:, :])
        ropes_sb = const.tile([128, NK], BF16, tag="ropes")
        nc.scalar.dma_start(out=ropes_sb, in_=ropes[:, :])
        kbt_sb = const.tile([128, 6], F32, tag="kbt")
        nc.scalar.dma_start(out=kbt_sb, in_=kbt[:, :])
        qg_sb = const.tile([128, 2 * H], F32, tag="qg")
        nc.scalar.dma_start(out=qg_sb, in_=qg[0:1, :].to_broadcast((128, 2 * H)))
        m0_sb = const.tile([128, 512], BF16, tag="m0")
        nc.scalar.dma_start(out=m0_sb, in_=m0[:, :])
        m2_sb = const.tile([128, 512], BF16, tag="m2")
        nc.scalar.dma_start(out=m2_sb, in_=m2[:, :])

        eps_t = const.tile([128, 1], F32, tag="eps")
        nc.vector.memset(eps_t, EPS)
        ones_b = const.tile([128, 128], BF16, tag="onesb")
        nc.vector.memset(ones_b, 1.0)
        hmask = const.tile([128, 65], BF16, tag="hmask")
        nc.vector.memset(hmask, 0.0)
        nc.vector.memset(hmask[0:64, 0:1], 1.0)
        nc.vector.memset(hmask[64:128, 64:65], 1.0)

        # ---- weights / activations (sync queue: wk, wv, xt; scalar: wq; gpsimd: wo)
        xt_sb, wq_sb, wk_sb, wv_sb, wo_sb = [], [], [], [], []
        for kt in range(8):
            sl = slice(kt * 128, kt * 128 + 128)
            t = big.tile([128, KVH * HD], BF16, tag=f"wk{kt}", name=f"wk{kt}")
            nc.sync.dma_start(out=t, in_=wk[sl, :])
            wk_sb.append(t)
            t = big.tile([128, KVH * HD], BF16, tag=f"wv{kt}", name=f"wv{kt}")
            nc.sync.dma_start(out=t, in_=wv[sl, :])
            wv_sb.append(t)
            t = big.tile([128, NK], BF16, tag=f"xt{kt}", name=f"xt{kt}")
            nc.sync.dma_start(out=t, in_=xt[sl, :])
            xt_sb.append(t)
        for kt in range(8):
            sl = slice(kt * 128, kt * 128 + 128)
            t = big.tile([128, DIM], BF16, tag=f"wq{kt}", name=f"wq{kt}")
            nc.scalar.dma_start(out=t, in_=wq[sl, :])
            wq_sb.append(t)
            t = big.tile([128, DIM], BF16, tag=f"wo{kt}", name=f"wo{kt}")
            nc.gpsimd.dma_start(out=t, in_=wo[sl, :])
            wo_sb.append(t)

        # ---- persistent SBUF ----
        ktp = [
            big.tile([128, NK], BF16, tag=f"ktp{j}", name=f"ktp{j}") for j in range(2)
        ]
        qtp = [
            big.tile([128, 4, NQ], BF16, tag=f"qtp{j}", name=f"qtp{j}")
            for j in range(2)
        ]
        v_sb = big.tile([128, 6, KVH, 2 * HD], BF16, tag="v")
        nc.vector.memset(v_sb[:, :, :, HD : 2 * HD], 1.0)
        yt_sb = big.tile([128, 8, NQ], BF16, tag="yt")

        def rope(v, dst, w, off=0):
            """v: [128, w] bf16 normalized (d-partition layout), covering kv
            positions [off, off+w).  Writes the rotated result to dst
            ([128, w], same-start)."""
            t1 = tp.tile([128, NK], BF16, tag="t1")
            nc.vector.tensor_mul(out=t1[:, 0:w], in0=v, in1=ropec_sb[:, off : off + w])
            u = tp.tile([128, NK], BF16, tag="u")
            nc.vector.tensor_mul(out=u[:, 0:w], in0=v, in1=ropes_sb[:, off : off + w])
            t2 = tp.tile([128, NK], BF16, tag="t2")
            for qtr in range(4):
                sp = (qtr ^ 1) * 32
                nc.vector.tensor_copy(
                    out=t2[qtr * 32 : qtr * 32 + 32, 0:w],
                    in_=u[sp : sp + 32, 0:w],
                )
            nc.vector.tensor_add(out=dst, in0=t1[:, 0:w], in1=t2[:, 0:w])

        # ---------------- K projection: kt-outer over 4 psum banks ----------
        psK = [psb.tile([128, 512], F32, tag="p512", name=f"psK{i}") for i in range(4)]
        for kt in range(8):
            for jp in range(2):
                for nh in range(2):
                    nc.tensor.matmul(
                        out=psK[jp * 2 + nh][:, 0:384],
                        lhsT=wk_sb[kt][:, jp * 128 : jp * 128 + 128],
                        rhs=xt_sb[kt][:, nh * 384 : nh * 384 + 384],
                        start=(kt == 0),
                        stop=(kt == 7),
                        skip_group_check=True,
                    )

        # ---------------- V projection: st-inner, double-buffered ----------
        for st in range(6):
            pv = psa.tile([128, 512], F32, tag="paux")
            for kt in range(8):
                nc.tensor.matmul(
                    out=pv[:, 0 : KVH * HD],
                    lhsT=xt_sb[kt][:, st * 128 : st * 128 + 128],
                    rhs=wv_sb[kt],
                    start=(kt == 0),
                    stop=(kt == 7),
                )
            nc.vector.tensor_copy(
                out=v_sb[:, st, :, 0:HD],
                in_=pv[:, 0 : KVH * HD].rearrange("p (g d) -> p g d", d=HD),
            )

        # ---- K norm + rope -> ktp[jp] ----
        for jp in range(2):
            sqK = sqp.tile([128, NK], BF16, tag="sq")
            for nh in range(2):
                nc.scalar.activation(
                    out=sqK[:, nh * 384 : nh * 384 + 384],
                    in_=psK[jp * 2 + nh][:, 0:384],
                    func=mybir.ActivationFunctionType.Square,
                )
            ssq = [pss.tile([65, 512], F32, tag="ps2", name=f"ssqK{i}") for i in range(2)]
            for nh in range(2):
                nc.tensor.matmul(
                    out=ssq[nh][0:65, 0:384],
                    lhsT=hmask,
                    rhs=sqK[:, nh * 384 : nh * 384 + 384],
                    start=True,
                    stop=True,
                )
            inv = invp.tile([128, NK], BF16, tag="inv")
            for nh in range(2):
                for row in range(2):
                    nc.scalar.activation(
                        out=inv[row * 64 : row * 64 + 1, nh * 384 : nh * 384 + 384],
                        in_=ssq[nh][row * 64 : row * 64 + 1, 0:384],
                        func=mybir.ActivationFunctionType.Sqrt,
                        bias=eps_t[row * 64 : row * 64 + 1, :],
                        scale=1.0 / HD,
                    )
            invb = invp.tile([128, NK], F32, tag="invb")
            for nh in range(2):
                ibp = pss.tile([128, 512], F32, tag="ps2", name="ibpK")
                nc.tensor.matmul(
                    out=ibp[0:64, 0:384],
                    lhsT=ones_b[0:1, 0:64],
                    rhs=inv[0:1, nh * 384 : nh * 384 + 384],
                    start=True,
                    stop=True,
                )
                nc.tensor.matmul(
                    out=ibp[64:128, 0:384],
                    lhsT=ones_b[64:65, 0:64],
                    rhs=inv[64:65, nh * 384 : nh * 384 + 384],
                    start=True,
                    stop=True,
                )
                nc.vector.reciprocal_approx_fast(
                    out=invb[:, nh * 384 : nh * 384 + 384], in_=ibp[:, 0:384]
                )
            kn = tp.tile([128, NK], BF16, tag="kn")
            for nh in range(2):
                nc.vector.tensor_mul(
                    out=kn[:, nh * 384 : nh * 384 + 384],
                    in0=psK[jp * 2 + nh][:, 0:384],
                    in1=invb[:, nh * 384 : nh * 384 + 384],
                )
            rope(kn, ktp[jp], NK)


        # ---------------- Q projection: two-stage pipeline ----------
        # stage A: proj matmuls + square (PE + scalar); stage B (issued one
        # tile later so the PE queue never waits on the norm chain): sumsq MM,
        # sqrt, broadcast MMs, approx-recip, normalize, rope, scatter.
        q_state = {}

        def q_stage_a(t):
            pq = psb.tile([128, 512], F32, tag="p512", name=f"pq{t}")
            for kt in range(8):
                nc.tensor.matmul(
                    out=pq,
                    lhsT=wq_sb[kt][:, t * 128 : t * 128 + 128],
                    rhs=xt_sb[kt][:, 256:768],
                    start=(kt == 0),
                    stop=(kt == 7),
                )
            sqQ = sqp.tile([128, NK], BF16, tag="sq")
            nc.scalar.activation(
                out=sqQ[:, 0:512],
                in_=pq,
                func=mybir.ActivationFunctionType.Square,
            )
            q_state[t] = (pq, sqQ)

        def q_stage_b(t):
            pq, sqQ = q_state.pop(t)
            ssq = pss.tile([65, 512], F32, tag="ps2")
            nc.tensor.matmul(
                out=ssq[0:65, :], lhsT=hmask, rhs=sqQ[:, 0:512], start=True, stop=True
            )
            inv = invp.tile([128, NK], BF16, tag="inv")
            for row in range(2):
                # 1/(qg*sqrt(m+eps)) = 1/sqrt(qg^2*m + qg^2*eps); qg_sb holds
                # qg^2/HD and qg^2*eps per head (broadcast to all partitions)
                p0 = row * 64
                h2 = 2 * (2 * t + row)
                nc.scalar.activation(
                    out=inv[p0 : p0 + 1, 0:512],
                    in_=ssq[p0 : p0 + 1, :],
                    func=mybir.ActivationFunctionType.Sqrt,
                    bias=qg_sb[p0 : p0 + 1, h2 + 1 : h2 + 2],
                    scale=qg_sb[p0 : p0 + 1, h2 : h2 + 1],
                )
            invb = invp.tile([128, NK], F32, tag="invb")
            ibp = pss.tile([128, 512], F32, tag="ps2", name="ibpQ")
            nc.tensor.matmul(
                out=ibp[0:64, :],
                lhsT=ones_b[0:1, 0:64],
                rhs=inv[0:1, 0:512],
                start=True,
                stop=True,
            )
            nc.tensor.matmul(
                out=ibp[64:128, :],
                lhsT=ones_b[64:65, 0:64],
                rhs=inv[64:65, 0:512],
                start=True,
                stop=True,
            )
            nc.vector.reciprocal_approx_fast(out=invb[:, 0:512], in_=ibp)
            qn = tp.tile([128, NK], BF16, tag="kn")
            nc.vector.tensor_mul(out=qn[:, 0:512], in0=pq, in1=invb[:, 0:512])
            # rotate then scatter the two heads into the qtp pair layout
            g = t // 2
            jp, pb = g // 2, (g % 2) * 64
            hh0 = 2 * (t % 2)
            qrot = tp.tile([128, NK], BF16, tag="qrot")
            rope(qn[:, 0:512], qrot[:, 0:512], 512, off=256)
            nc.vector.tensor_copy(
                out=qtp[jp][pb : pb + 64, hh0, :], in_=qrot[0:64, 0:512]
            )
            nc.vector.tensor_copy(
                out=qtp[jp][pb : pb + 64, hh0 + 1, :], in_=qrot[64:128, 0:512]
            )

        q_stage_a(0)
        for t in range(1, 8):
            q_stage_a(t)
            q_stage_b(t - 1)
        q_stage_b(7)

        # ---------------- attention ----------
        def attention(jp, qb):
            atts = []
            for t in range(3):
                kw = slice(qb * 128 + t * 128, qb * 128 + t * 128 + 128)
                att = attp.tile([128, 1024], BF16, tag="att")
                kc = qb + t
                for half in range(2):
                    ps_ = psb.tile([128, 512], F32, tag="p512")
                    nc.tensor.matmul(
                        out=ps_,
                        lhsT=ktp[jp][half * 64 : half * 64 + 64, kw],
                        rhs=qtp[jp][
                            half * 64 : half * 64 + 64, :, qb * 128 : qb * 128 + 128
                        ],
                        start=True,
                        stop=True,
                    )
                    nc.scalar.activation(
                        out=att[:, half * 512 : half * 512 + 512],
                        in_=ps_,
                        func=mybir.ActivationFunctionType.Exp,
                        bias=kbt_sb[:, kc : kc + 1],
                    )
                if t == 0:
                    for half in range(2):
                        nc.vector.tensor_mul(
                            out=att[:, half * 512 : half * 512 + 512],
                            in0=att[:, half * 512 : half * 512 + 512],
                            in1=m0_sb,
                        )
                if t == 2:
                    for half in range(2):
                        nc.vector.tensor_mul(
                            out=att[:, half * 512 : half * 512 + 512],
                            in0=att[:, half * 512 : half * 512 + 512],
                            in1=m2_sb,
                        )
                if debug and jp == 0 and qb == 1:
                    nc.sync.dma_start(
                        out=d_att[:, t * 1024 : t * 1024 + 1024], in_=att
                    )
                atts.append(att)
            for half in range(2):
                g = 2 * jp + half
                psy = psa.tile([128, 512], F32, tag="paux")
                for t in range(3):
                    nc.tensor.matmul(
                        out=psy,
                        lhsT=v_sb[:, qb + t, g, :],
                        rhs=atts[t][:, half * 512 : half * 512 + 512],
                        start=(t == 0),
                        stop=(t == 2),
                    )
                rb = rp.tile([128, 512], F32, tag="rb")
                nc.vector.reciprocal_approx_fast(out=rb, in_=psy)
                nc.vector.tensor_copy(out=rb[0:64, :], in_=rb[64:128, :])
                for hh in range(4):
                    h = g * 4 + hh
                    pair, lo = h // 2, (h % 2) * 64
                    nc.vector.tensor_mul(
                        out=yt_sb[lo : lo + 64, pair, qb * 128 : qb * 128 + 128],
                        in0=psy[0:64, hh * 128 : hh * 128 + 128],
                        in1=rb[lo : lo + 64, hh * 128 : hh * 128 + 128],
                    )

        if debug:
            for j in range(2):
                nc.sync.dma_start(out=d_ktp[j * 128 : j * 128 + 128, :], in_=ktp[j])
                nc.sync.dma_start(
                    out=d_qtp[j * 128 : j * 128 + 128, :],
                    in_=qtp[j].rearrange("p h q -> p (h q)"),
                )
            nc.sync.dma_start(out=d_v[:, :], in_=v_sb.rearrange("p a b c -> p (a b c)"))

        for qb in range(4):
            attention(0, qb)

        def out_proj(qb):
            ob = obp.tile([128, DIM], F32, tag="ob")
            for half in range(2):
                po = psb.tile([128, 512], F32, tag="p512", name=f"po{qb}{half}")
                for p in range(8):
                    nc.tensor.matmul(
                        out=po,
                        lhsT=yt_sb[:, p, qb * 128 : qb * 128 + 128],
                        rhs=wo_sb[p][:, half * 512 : half * 512 + 512],
                        start=(p == 0),
                        stop=(p == 7),
                    )
                nc.scalar.copy(out=ob[:, half * 512 : half * 512 + 512], in_=po)
            nc.sync.dma_start(out=out[qb * 128 : qb * 128 + 128, :], in_=ob)

        attention(1, 0)
        for qb in range(1, 4):
            attention(1, qb)
            out_proj(qb - 1)
        out_proj(3)
        if debug:
            nc.sync.dma_start(out=d_yt[:, :], in_=yt_sb.rearrange("p h q -> p (h q)"))

    nc.finalize()
    return nc


def _host_inputs(x, Wq, Wk, Wv, Wo, q_gain, pair_mix):
    """Build the 8 per-core input maps."""
    x = np.asarray(x, np.float32)
    Wq = np.asarray(Wq, np.float32)
    Wk = np.asarray(Wk, np.float32)
    Wv = np.asarray(Wv, np.float32)
    Wo = np.asarray(Wo, np.float32)
    q_gain = np.asarray(q_gain, np.float32)
    pair_mix = np.asarray(pair_mix, np.float32)

    # fold pair mixing into Wo:  out = y_mix @ Wo.T,  y_mix = y @ M.T  =>  Wo' = Wo @ M
    M = np.zeros((DIM, DIM), np.float32)
    eye = np.eye(HD, dtype=np.float32)
    for p in range(H // 2):
        for o in range(2):
            for i in range(2):
                ho, hi = 2 * p + o, 2 * p + i
                M[ho * HD : ho * HD + HD, hi * HD : hi * HD + HD] = (
                    pair_mix[p, o, i] * eye
                )
    woT = np.ascontiguousarray((Wo @ M).T)

    wqT = np.ascontiguousarray(Wq.T)
    wkT = np.ascontiguousarray(Wk.T)
    wvT = np.ascontiguousarray(Wv.T)
    qg2 = (q_gain / math.sqrt(HD)).astype(np.float32) ** 2
    qgt = np.zeros((1, 2 * H), np.float32)
    qgt[0, 0::2] = 1.0 / (HD * qg2)
    qgt[0, 1::2] = EPS / qg2

    inv_freq = 1.0 / (ROPE_BASE ** (np.arange(0, HD, 2, dtype=np.float32) / HD))

    ql = np.arange(128)
    m0_ = (ql[:, None] >= ql[None, :] + 1).astype(np.float32)  # kl >= ql+1
    m2_ = (ql[:, None] <= ql[None, :]).astype(np.float32)      # kl <= ql
    m0t = np.ascontiguousarray(np.tile(m0_, (1, 4)))
    m2t = np.ascontiguousarray(np.tile(m2_, (1, 4)))

    import ml_dtypes

    bf = ml_dtypes.bfloat16
    wqT, wkT, wvT, woT = (a.astype(bf) for a in (wqT, wkT, wvT, woT))
    m0t, m2t = m0t.astype(bf), m2t.astype(bf)

    # rope tables in [d-partition, pos] layout, head block repeated twice;
    # rows 0-31: +sin, rows 32-63: -sin (sign folded)
    r = np.arange(128)
    freqs_row = inv_freq[r % 32]                       # [128]
    sign = np.where((r % 64) < 32, -1.0, 1.0).astype(np.float32)

    in_maps = []
    for core in range(NCORES):
        b, c = core // 4, core % 4
        ks = 512 * c - 256
        xc = np.zeros((NK, DIM), np.float32)
        lo = max(0, ks)
        xc[lo - ks :] = x[b, lo : ks + NK]
        pos = (ks + np.arange(NK, dtype=np.float32))[None, :]    # [1, NK]
        ang = freqs_row[:, None] * pos                           # [128, NK]
        ropec_ = np.cos(ang).astype(bf)
        ropes_ = (np.sin(ang) * sign[:, None]).astype(bf)
        kbt_ = np.zeros((128, 6), np.float32)
        if c == 0:
            kbt_[:, 0:2] = -30000.0
        in_maps.append(
            {
                "xt": np.ascontiguousarray(xc.T).astype(bf),
                "wq": wqT,
                "wk": wkT,
                "wv": wvT,
                "wo": woT,
                "ropec": np.ascontiguousarray(ropec_),
                "ropes": np.ascontiguousarray(ropes_),
                "kbt": kbt_,
                "qg": qgt,
                "m0": m0t,
                "m2": m2t,
            }
        )
    return in_maps


def kernel(x, Wq, Wk, Wv, Wo, q_gain, pair_mix):
    global _BUILT
    from concourse.bass_utils import run_bass_kernel_spmd

    if _BUILT is None:
        _BUILT = _build()
    in_maps = _host_inputs(x, Wq, Wk, Wv, Wo, q_gain, pair_mix)
    res = run_bass_kernel_spmd(_BUILT, in_maps, list(range(NCORES)))
    out = np.empty((B, S, DIM), np.float32)
    for core in range(NCORES):
        b, c = core // 4, core % 4
        out[b, 512 * c : 512 * c + 512, :] = res.results[core]["out"]
    return out
